# revision 17
# baseline (speedup 1.0000x reference)
"""Trainium2 Bass kernel for nn_CGNN_88038239634099 (GNN message passing).

Math: the edge gather/scatter-add over a fixed 64-node graph is a dense
64x64 adjacency matmul (A[dst,src] += w).  Per layer:
    h <- relu(h + A @ (h @ W_l + b_l))
Everything becomes dense matmuls over B=4096 independent samples.

v3 layout (all matmuls bf16, PSUM fp32), h feature-major
h[feat=128 partitions, token], token t = b*64 + n:

  Layer 1 is fully rank-collapsed: with u = W1^T w_enc,
  v = W1^T b_enc + b1, rowA = A @ 1,
      pre1 = h0 + A(h0 W1 + b1)
           = x (x) w_enc + 1 (x) b_enc + (Ax) (x) u + rowA (x) v
  i.e. ONE K=4 const-stationary ap-512 matmul per 512-token group from a
  z = [x; 1; Ax; rowA] streaming bundle.  The Ax row is built on device:
  one mtokT matmul over token-major x, 2 PE transposes, and a
  partition->row flatten DMA (~2us once per iteration).

  Layers 2,3 per 512-token group:
    mm1 [PE]  : p1[tok,feat] = lhsT(h_blk).T @ W_l        (4x ap-128 flip)
    move[DVE] : hn = p1 + b_l   (PSUM->SBUF bf16)
    resid[PE] : p2 = I.T @ h_grp  (ap-512 const, start=True opens bank)
    mm2 [PE]  : p2 += lhsT(hn_blk).T @ MtokT  (4x ap-128 flip, accumulate)
    relu[ACT] : h_grp = Relu(p2)
  with MtokT = kron(I2, A.T).  Emission is software-pipelined (skew 1);
  resid is issued before mm2 so PE works while DVE finishes the move.
  Classifier contracts over (n,h) via 64 accumulating ap-512 matmuls.

Sharding: data-parallel over batch, 512 samples per core, 8 cores.
"""

import sys

if "/opt/trn_rl_repo" not in sys.path:
    sys.path.insert(0, "/opt/trn_rl_repo")

import numpy as np
import ml_dtypes

B, N, H, L, O = 4096, 64, 128, 3, 2
NCORES = 8
B_LOC = B // NCORES          # 512 samples per core
BC = 512                     # samples per chunk
NCHUNK = B_LOC // BC         # 1
T = BC * N                   # 32768 tokens per chunk
NBLK = T // 128              # 256 blocks of 128 tokens
NGRP = NBLK // 4             # 64 groups of 4 blocks (512 tokens)

_CACHE = {}
RESID_FIRST = False


def _build_module(repeat=1, ablate=()):
    """Build + compile the Bass/Tile module (same SPMD program on 8 cores).

    repeat>1 wraps the compute in a hardware loop that redoes the same
    work; used only for slope-based timing (outputs unchanged).
    ablate: subset of {'dve','act'} — drop those engines' per-group ops
    (breaks correctness; timing experiments only)."""
    import concourse.bass as bass
    import concourse.tile as tile
    from concourse import bacc, mybir

    f32 = mybir.dt.float32
    bf16 = mybir.dt.bfloat16
    AF = mybir.ActivationFunctionType
    ALU = mybir.AluOpType

    nc = bacc.Bacc(
        "TRN2",
        target_bir_lowering=False,
        debug=False,
        enable_asserts=False,
        num_devices=NCORES,
    )

    x_d = nc.dram_tensor("x_loc", [NCHUNK, 2, T], bf16, kind="ExternalInput").ap()
    xtm_d = nc.dram_tensor("x_tm", [NCHUNK, 128, NBLK], bf16, kind="ExternalInput").ap()
    rowa_d = nc.dram_tensor("rowa", [1, T], bf16, kind="ExternalInput").ap()
    mtokT_d = nc.dram_tensor("mtokT", [128, 128], bf16, kind="ExternalInput").ap()
    zw_d = nc.dram_tensor("zw", [4, 128], bf16, kind="ExternalInput").ap()
    wl_d = nc.dram_tensor("wl", [L - 1, 128, 128], bf16, kind="ExternalInput").ap()
    blrep_d = nc.dram_tensor("blrep", [L - 1, 128, 512], bf16, kind="ExternalInput").ap()
    i128_d = nc.dram_tensor("i128", [128, 128], bf16, kind="ExternalInput").ap()
    wc1_d = nc.dram_tensor("wc1", [N * H, H], bf16, kind="ExternalInput").ap()
    bc1_d = nc.dram_tensor("bc1", [128, 1], f32, kind="ExternalInput").ap()
    wc2_d = nc.dram_tensor("wc2", [128, O], bf16, kind="ExternalInput").ap()
    bc2_d = nc.dram_tensor("bc2", [O, 1], f32, kind="ExternalInput").ap()
    out_d = nc.dram_tensor("out_loc", [NCHUNK, BC, O], f32, kind="ExternalOutput").ap()
    # scratch for the partition->row flatten of the Ax row (DRAM roundtrip)
    axrow_d = nc.dram_tensor("axrow_scratch", [1, T], bf16, kind="Internal").ap()

    with tile.TileContext(nc) as tc:
        with (
            tc.tile_pool(name="consts", bufs=1) as cpool,
            tc.tile_pool(name="h", bufs=1) as hpool,
            tc.tile_pool(name="z", bufs=1) as zpool,
            tc.tile_pool(name="hn", bufs=3) as hn_pool,
            tc.tile_pool(name="tmp", bufs=3) as tmp_pool,
            tc.tile_pool(name="hid", bufs=1) as hid_pool,
            tc.tile_pool(name="ps1", bufs=3, space=bass.MemorySpace.PSUM) as ps1_pool,
            tc.tile_pool(name="ps2", bufs=3, space=bass.MemorySpace.PSUM) as ps2_pool,
            tc.tile_pool(name="psm", bufs=1, space=bass.MemorySpace.PSUM) as psm_pool,
        ):
            # ---- load constants into SBUF ----
            c_mtokT = cpool.tile([128, 128], bf16, tag="mtokT")
            nc.sync.dma_start(c_mtokT[:], mtokT_d[:])
            c_zw = cpool.tile([4, 128], bf16, tag="zw")
            nc.sync.dma_start(c_zw[:], zw_d[:])
            c_wl = []
            c_bl = []
            for l in range(L - 1):
                wt = cpool.tile([128, 128], bf16, tag=f"wl{l}")
                nc.sync.dma_start(wt[:], wl_d[l])
                c_wl.append(wt)
                bt = cpool.tile([128, 512], bf16, tag=f"bl{l}")
                nc.sync.dma_start(bt[:], blrep_d[l])
                c_bl.append(bt)
            c_i128 = cpool.tile([128, 128], bf16, tag="i128")
            nc.sync.dma_start(c_i128[:], i128_d[:])
            c_wc1 = cpool.tile([128, N * 128], bf16, tag="wc1")
            for n in range(N):
                nc.sync.dma_start(
                    c_wc1[:, n * 128 : (n + 1) * 128],
                    wc1_d[n * 128 : (n + 1) * 128, :],
                )
            c_bc1 = cpool.tile([128, 1], f32, tag="bc1")
            nc.sync.dma_start(c_bc1[:], bc1_d[:])
            c_wc2 = cpool.tile([128, O], bf16, tag="wc2")
            nc.sync.dma_start(c_wc2[:], wc2_d[:])
            c_bc2 = cpool.tile([O, 1], f32, tag="bc2")
            nc.sync.dma_start(c_bc2[:], bc2_d[:])

            h = hpool.tile([128, T], bf16, tag="h")
            if ablate:
                # timing-only variants may never write h; make reads legal
                nc.gpsimd.memset(h[:], 0.0)

            def compute():
                for c in range(NCHUNK):
                    _chunk(c)

            def _chunk(c):
                # ---- build z = [x; 1; Ax; rowA] bundle ----
                z = zpool.tile([4, T], bf16, tag="z")
                nc.sync.dma_start(z[0:2, :], x_d[c])
                nc.sync.dma_start(z[3:4, :], rowa_d[:])
                xtm = zpool.tile([128, NBLK], bf16, tag="xtm")
                nc.sync.dma_start(xtm[:], xtm_d[c])
                pax = psm_pool.tile([128, NBLK], f32, tag="psm")
                nc.tensor.matmul(pax[:], c_mtokT[:], xtm[:], start=True, stop=True)
                axs = zpool.tile([128, NBLK], bf16, tag="axs")
                nc.vector.tensor_copy(axs[:], pax[:])
                tts = zpool.tile([128, NBLK], bf16, tag="tts")
                for half in range(2):
                    ptr = psm_pool.tile([128, 128], bf16, tag="psm2")
                    nc.tensor.transpose(
                        ptr[:], axs[:, half * 128 : (half + 1) * 128], c_i128[:]
                    )
                    nc.vector.tensor_copy(
                        tts[:, half * 128 : (half + 1) * 128], ptr[:]
                    )
                    # partition->row flatten via DRAM (SBUF APs can't cross
                    # the partition dim; DRAM APs can)
                    nc.sync.dma_start(
                        axrow_d[:, half * 16384 : (half + 1) * 16384].rearrange(
                            "o (b t) -> (o b) t", b=128
                        ),
                        tts[:, half * 128 : (half + 1) * 128],
                    )
                    nc.sync.dma_start(
                        z[2 : 3, half * 16384 : (half + 1) * 16384],
                        axrow_d[:, half * 16384 : (half + 1) * 16384],
                    )

                # ---- layer 1: one K=4 const-stationary matmul per group ----
                def l1_group(g):
                    g0 = g * 512
                    p2 = ps2_pool.tile([128, 512], f32, tag="p2")
                    nc.tensor.matmul(
                        p2[:], c_zw[:], z[:, g0 : g0 + 512], start=True, stop=True
                    )
                    if "act" not in ablate:
                        nc.scalar.activation(h[:, g0 : g0 + 512], p2[:], AF.Relu)

                for g in range(NGRP):
                    l1_group(g)

                # ---- layers 2,3 ----
                def mm1(l, g):
                    p1 = ps1_pool.tile([128, 512], f32, tag="p1")
                    for j in range(4):
                        blk = g * 512 + j * 128
                        nc.tensor.matmul(
                            p1[:, j * 128 : (j + 1) * 128],
                            h[:, blk : blk + 128],
                            c_wl[l][:],
                            start=True,
                            stop=True,
                        )
                    return p1

                def stage2(l, g, p1):
                    g0 = g * 512
                    if "dve" in ablate:
                        hn = c_bl[l]
                    else:
                        hn = hn_pool.tile([128, 512], bf16, tag="hn")
                        nc.vector.tensor_tensor(hn[:], p1[:], c_bl[l][:], ALU.add)
                    p2 = ps2_pool.tile([128, 512], f32, tag="p2")
                    for j in range(4):
                        nc.tensor.matmul(
                            p2[:, j * 128 : (j + 1) * 128],
                            hn[:, j * 128 : (j + 1) * 128],
                            c_mtokT[:],
                            start=True,
                            stop=True,
                        )
                    # residual add off the PE: tmp = p2 + h on DVE/Pool
                    tmp = tmp_pool.tile([128, 512], bf16, tag="tmp")
                    if "dve" not in ablate:
                        nc.vector.tensor_tensor(
                            tmp[:], p2[:], h[:, g0 : g0 + 512], ALU.add
                        )
                    if "act" not in ablate:
                        nc.scalar.activation(h[:, g0 : g0 + 512], tmp[:], AF.Relu)

                for l in range(L - 1):
                    prev = None
                    for g in range(NGRP):
                        p1 = mm1(l, g)
                        if prev is not None:
                            stage2(l, prev[0], prev[1])
                        prev = (g, p1)
                    stage2(l, prev[0], prev[1])

                # ---- classifier: hidden = relu(h_flat @ Wc1 + bc1) ----
                h3 = h[:].rearrange("p (b n) -> p n b", n=N)  # [128, N, BC]
                pc = psm_pool.tile([128, BC], f32, tag="psm")
                for n in range(N):
                    nc.tensor.matmul(
                        pc[:],
                        c_wc1[:, n * 128 : (n + 1) * 128],
                        h3[:, n, :],
                        start=(n == 0),
                        stop=(n == N - 1),
                    )
                hid = hid_pool.tile([128, BC], bf16, tag="hid")
                nc.scalar.activation(hid[:], pc[:], AF.Relu, bias=c_bc1[:])

                # ---- logits = hidden @ Wc2 + bc2 ----
                po = psm_pool.tile([O, BC], f32, tag="psm2")
                nc.tensor.matmul(po[:], c_wc2[:], hid[:], start=True, stop=True)
                lg = hid_pool.tile([O, BC], f32, tag="lg")
                nc.scalar.activation(lg[:], po[:], AF.Identity, bias=c_bc2[:])
                nc.sync.dma_start(out_d[c].rearrange("b o -> o b"), lg[:])

            if repeat == 1:
                compute()
            else:
                with tc.For_i(0, repeat, 1):
                    compute()

    nc.compile()
    return nc


def _precompute_consts(edge_index, edge_attr, w_enc, b_enc, W_layers, b_layers,
                       Wc1, bc1, Wc2, bc2):
    bf = ml_dtypes.bfloat16
    src = np.asarray(edge_index[0], dtype=np.int64)
    dst = np.asarray(edge_index[1], dtype=np.int64)
    w = np.asarray(edge_attr, dtype=np.float32)[:, 0]
    A = np.zeros((N, N), dtype=np.float32)
    np.add.at(A, (dst, src), w)
    mtokT = np.kron(np.eye(2, dtype=np.float32), A.T).astype(np.float32)
    rowA = A.sum(axis=1)  # A @ 1
    bl = np.asarray(b_layers, dtype=np.float32)
    W = np.asarray(W_layers, dtype=np.float32)
    u = W[0].T @ np.asarray(w_enc, np.float32)
    v = W[0].T @ np.asarray(b_enc, np.float32) + bl[0]
    zw = np.stack(
        [np.asarray(w_enc, np.float32), np.asarray(b_enc, np.float32), u, v], 0
    )
    blrep = np.broadcast_to(
        np.tile(bl[1:], (1, 4))[:, None, :], (L - 1, 128, 512)
    )
    consts = {
        "mtokT": np.ascontiguousarray(mtokT).astype(bf),
        "zw": np.ascontiguousarray(zw).astype(bf),
        "rowa": np.ascontiguousarray(np.tile(rowA, T // N)[None, :]).astype(bf),
        "i128": np.eye(128, dtype=np.float32).astype(bf),
        "wl": np.ascontiguousarray(W[1:]).astype(bf),
        "blrep": np.ascontiguousarray(blrep).astype(bf),
        "wc1": np.ascontiguousarray(np.asarray(Wc1, dtype=np.float32)).astype(bf),
        "bc1": np.ascontiguousarray(np.asarray(bc1, np.float32).reshape(128, 1)),
        "wc2": np.ascontiguousarray(np.asarray(Wc2, dtype=np.float32)).astype(bf),
        "bc2": np.ascontiguousarray(np.asarray(bc2, np.float32).reshape(O, 1)),
    }
    return consts


def _get_nc(repeat=1, ablate=()):
    key = ("nc", repeat, tuple(ablate))
    if key not in _CACHE:
        _CACHE[key] = _build_module(repeat, ablate)
    return _CACHE[key]


def _make_in_maps(inputs):
    consts = _precompute_consts(
        inputs["edge_index"], inputs["edge_attr"], inputs["w_enc"],
        inputs["b_enc"], inputs["W_layers"], inputs["b_layers"],
        inputs["Wc1"], inputs["bc1"], inputs["Wc2"], inputs["bc2"],
    )
    x = np.asarray(inputs["x"], dtype=np.float32)
    in_maps = []
    for core in range(NCORES):
        xc = x[core * B_LOC : (core + 1) * B_LOC].reshape(NCHUNK, 1, T)
        xc = np.concatenate([xc, np.ones_like(xc)], axis=1)  # row 1 = ones
        xtm = (
            x[core * B_LOC : (core + 1) * B_LOC]
            .reshape(NCHUNK, NBLK, 128)
            .transpose(0, 2, 1)
        )
        m = {
            "x_loc": np.ascontiguousarray(xc).astype(ml_dtypes.bfloat16),
            "x_tm": np.ascontiguousarray(xtm).astype(ml_dtypes.bfloat16),
        }
        m.update(consts)
        in_maps.append(m)
    return in_maps


def _run(inputs, trace=False):
    """inputs: full unsharded dict. Returns (logits [B,O], BassKernelResults)."""
    from concourse import bass_utils

    nc = _get_nc()
    in_maps = _make_in_maps(inputs)
    res = bass_utils.run_bass_kernel_spmd(
        nc, in_maps, core_ids=list(range(NCORES)), trace=trace
    )
    out = np.concatenate(
        [res.results[c]["out_loc"].reshape(B_LOC, O) for c in range(NCORES)], axis=0
    )
    return out, res


def kernel(**inputs):
    out, _ = _run(inputs, trace=False)
    return out
